# revision 6
# baseline (speedup 1.0000x reference)
"""Trainium2 Bass kernel for an 8-expert top-2 MoE SwiGLU FFN layer.

Sharding: expert-parallel over 8 NeuronCores (one expert per core).
Host side (the shard/unshard steps): token routing (softmax top-2) to build
the per-expert dispatch, gather/pad token batches to a fixed capacity,
scatter-add the weighted expert outputs back. Device side: per-expert dense
SwiGLU FFN over the dispatched tokens plus the router-logits matmul.

Everything on-device works in a transposed [feature, token] layout so all
three matmuls consume weights in their natural [in, out] layout with tokens
as the moving operand; no on-chip transposes are needed.
"""

import numpy as np
import ml_dtypes

import concourse.bass as bass
import concourse.bacc as bacc
import concourse.mybir as mybir
from concourse.tile import TileContext
from concourse.bass_utils import run_bass_kernel_spmd

B, S, H = 4, 2048, 1024
F = 4 * H            # 4096
E = 8
TOP_K = 2
T = B * S            # 8192
P = 128
HK = H // P          # 8
FM = F // P          # 32
TSLICE = T // E      # 1024 router-logit tokens per core

# Per-expert token capacity. Seed-0 inputs route at most 2182 tokens to one
# expert; 2304 = 9*256 leaves margin and keeps 256-wide matmul tiles legal.
C = 2304
OUTER = [(0, 1024), (1024, 1024), (2048, 256)]

_NC = None
LAST_RESULTS = None


def _build_bass():
    nc = bacc.Bacc("TRN2", target_bir_lowering=False)
    f32 = mybir.dt.float32
    f32r = mybir.dt.float32r
    bf16 = mybir.dt.bfloat16
    Silu = mybir.ActivationFunctionType.Silu

    xT = nc.dram_tensor("xT", [H, C], f32r, kind="ExternalInput")
    wg = nc.dram_tensor("wg", [H, F], f32r, kind="ExternalInput")
    wu = nc.dram_tensor("wu", [H, F], f32r, kind="ExternalInput")
    wd = nc.dram_tensor("wd", [F, H], bf16, kind="ExternalInput")
    cw = nc.dram_tensor("cw", [P, C], f32, kind="ExternalInput")
    xTs = nc.dram_tensor("xTs", [H, TSLICE], f32, kind="ExternalInput")
    wgr = nc.dram_tensor("wgr", [H, E], f32, kind="ExternalInput")
    yT = nc.dram_tensor("yT", [H, C], f32, kind="ExternalOutput")
    lgT = nc.dram_tensor("lgT", [E, TSLICE], f32, kind="ExternalOutput")

    xT_r = xT.rearrange("(k p) t -> p k t", p=P)
    wg_r = wg.rearrange("(k p) f -> p k f", p=P)
    wu_r = wu.rearrange("(k p) f -> p k f", p=P)
    wd_r = wd.rearrange("(m p) h -> p m h", p=P)
    xTs_r = xTs.rearrange("(k p) t -> p k t", p=P)
    wgr_r = wgr.rearrange("(k p) e -> p k e", p=P)
    yT_r = yT.rearrange("(j p) t -> p j t", p=P)

    with TileContext(nc) as tc:
        # Router logits for this core's 1024-token slice: lgT = wgr.T @ xTs.
        with (
            tc.tile_pool(name="router", bufs=2) as rp,
            tc.tile_pool(name="router_ps", bufs=2, space="PSUM") as rps,
        ):
            wgr_sb = rp.tile([P, HK, E], f32, tag="wgr")
            nc.sync.dma_start(out=wgr_sb, in_=wgr_r)
            xTs_sb = rp.tile([P, HK, TSLICE], f32, tag="xTs")
            nc.sync.dma_start(out=xTs_sb, in_=xTs_r)
            for n0 in range(0, TSLICE, 512):
                ps = rps.tile([E, 512], f32, tag="rps")
                for k in range(HK):
                    nc.tensor.matmul(
                        ps,
                        lhsT=wgr_sb[:, k, :],
                        rhs=xTs_sb[:, k, n0 : n0 + 512],
                        start=(k == 0),
                        stop=(k == HK - 1),
                    )
                lsb = rp.tile([E, 512], f32, tag="lsb")
                nc.vector.tensor_copy(out=lsb, in_=ps)
                nc.sync.dma_start(out=lgT[:, n0 : n0 + 512], in_=lsb)

        # SwiGLU FFN over the dispatched token batch, in [feature, token]
        # layout: h = silu(wg.T @ xT) * (wu.T @ xT); yT = (wd.T @ h) * cw.
        with (
            tc.tile_pool(name="xp", bufs=2) as xp,
            tc.tile_pool(name="wp", bufs=3) as wp,
            tc.tile_pool(name="hp", bufs=1) as hp,
            tc.tile_pool(name="sp", bufs=3) as sp,
            tc.tile_pool(name="op", bufs=3) as op,
            tc.tile_pool(name="cwp", bufs=1) as cwp,
            tc.tile_pool(name="gups", bufs=2, space="PSUM") as gups,
            tc.tile_pool(name="ops", bufs=2, space="PSUM") as ops,
        ):
            cw_sb = cwp.tile([P, C], f32, tag="cw")
            nc.sync.dma_start(out=cw_sb, in_=cw.ap())

            for c0, cn in OUTER:
                ntiles = [(o, min(512, cn - o)) for o in range(0, cn, 512)]
                xT_sb = xp.tile([P, HK, 1024], f32r, tag="xT")
                nc.sync.dma_start(out=xT_sb[:, :, :cn], in_=xT_r[:, :, c0 : c0 + cn])
                h_sb = hp.tile([P, FM, 1024], bf16, tag="h")

                for m in range(FM):
                    wg_sb = wp.tile([P, HK, P], f32r, tag="wg")
                    nc.sync.dma_start(out=wg_sb, in_=wg_r[:, :, m * P : (m + 1) * P])
                    wu_sb = wp.tile([P, HK, P], f32r, tag="wu")
                    nc.sync.dma_start(out=wu_sb, in_=wu_r[:, :, m * P : (m + 1) * P])
                    for no, nn in ntiles:
                        gps = gups.tile([P, 512], f32, tag="g", name="gps")[:, :nn]
                        ups = gups.tile([P, 512], f32, tag="u", name="ups")[:, :nn]
                        for k in range(HK):
                            nc.tensor.matmul(
                                gps,
                                lhsT=wg_sb[:, k, :],
                                rhs=xT_sb[:, k, no : no + nn],
                                start=(k == 0),
                                stop=(k == HK - 1),
                            )
                        for k in range(HK):
                            nc.tensor.matmul(
                                ups,
                                lhsT=wu_sb[:, k, :],
                                rhs=xT_sb[:, k, no : no + nn],
                                start=(k == 0),
                                stop=(k == HK - 1),
                            )
                        sg = sp.tile([P, 512], f32, tag="sg", name="sg")[:, :nn]
                        nc.scalar.activation(sg, gps, Silu)
                        nc.vector.tensor_mul(
                            out=h_sb[:, m, no : no + nn], in0=sg, in1=ups
                        )

                for j in range(HK):
                    wd_sb = wp.tile([P, FM, P], bf16, tag="wd")
                    nc.sync.dma_start(out=wd_sb, in_=wd_r[:, :, j * P : (j + 1) * P])
                    for no, nn in ntiles:
                        ops_t = ops.tile([P, 512], f32, tag="o", name="opst")[:, :nn]
                        for m in range(FM):
                            nc.tensor.matmul(
                                ops_t,
                                lhsT=wd_sb[:, m, :],
                                rhs=h_sb[:, m, no : no + nn],
                                start=(m == 0),
                                stop=(m == FM - 1),
                            )
                        ob = op.tile([P, 512], f32, tag="ob", name="ob")[:, :nn]
                        nc.vector.tensor_mul(
                            out=ob, in0=ops_t, in1=cw_sb[:, c0 + no : c0 + no + nn]
                        )
                        nc.sync.dma_start(
                            out=yT_r[:, j, c0 + no : c0 + no + nn], in_=ob
                        )
    nc.compile()
    return nc


def _get_nc():
    global _NC
    if _NC is None:
        _NC = _build_bass()
    return _NC


def kernel(hidden_states, w_gate, w_gate_proj, w_up_proj, w_down_proj):
    global LAST_RESULTS
    x = np.ascontiguousarray(np.asarray(hidden_states, dtype=np.float32).reshape(T, H))
    wgate = np.ascontiguousarray(np.asarray(w_gate, dtype=np.float32))

    # Routing decision (host — determines the expert-parallel dispatch).
    logits = x @ wgate
    mx = logits.max(-1, keepdims=True)
    ex = np.exp(logits - mx)
    probs = ex / ex.sum(-1, keepdims=True)
    top_i = np.argsort(-probs, axis=-1, kind="stable")[:, :TOP_K]
    top_w = np.take_along_axis(probs, top_i, axis=-1)
    top_w = top_w / top_w.sum(-1, keepdims=True)

    in_maps = []
    idx_list = []
    cnt_list = []
    for e in range(E):
        sel = top_i == e
        tok = np.nonzero(sel.any(-1))[0]
        cnt = len(tok)
        assert cnt <= C, f"expert {e} overflows capacity: {cnt} > {C}"
        w_tok = np.where(sel[tok, 0], top_w[tok, 0], top_w[tok, 1]).astype(np.float32)
        xTe = np.zeros((H, C), np.float32)
        xTe[:, :cnt] = x[tok].T
        cwv = np.zeros((C,), np.float32)
        cwv[:cnt] = w_tok
        in_maps.append(
            {
                "xT": xTe,
                "wg": np.ascontiguousarray(np.asarray(w_gate_proj[e], np.float32)),
                "wu": np.ascontiguousarray(np.asarray(w_up_proj[e], np.float32)),
                "wd": np.ascontiguousarray(np.asarray(w_down_proj[e], np.float32)).astype(
                    ml_dtypes.bfloat16
                ),
                "cw": np.ascontiguousarray(np.broadcast_to(cwv, (P, C))),
                "xTs": np.ascontiguousarray(x[e * TSLICE : (e + 1) * TSLICE].T),
                "wgr": wgate,
            }
        )
        idx_list.append(tok)
        cnt_list.append(cnt)

    nc = _get_nc()
    res = run_bass_kernel_spmd(nc, in_maps, core_ids=list(range(E)))
    LAST_RESULTS = res

    out = np.zeros((T, H), np.float32)
    lg = np.zeros((T, E), np.float32)
    for e in range(E):
        yTe = np.asarray(res.results[e]["yT"])  # [H, C]
        out[idx_list[e]] += yTe.T[: cnt_list[e]]
        lg[e * TSLICE : (e + 1) * TSLICE] = np.asarray(res.results[e]["lgT"]).T
    return out.reshape(B, S, H), lg


# revision 15
# speedup vs baseline: 1.1057x; 1.1057x over previous
"""Trainium2 Bass kernel for an 8-expert top-2 MoE SwiGLU FFN layer.

Sharding: expert-parallel over 8 NeuronCores (one expert per core).
Host side (the shard/unshard steps): token routing (softmax top-2) to build
the per-expert dispatch, gather/pad token batches to a fixed capacity,
scatter-add the weighted expert outputs back. Device side: per-expert dense
SwiGLU FFN over the dispatched tokens plus the router-logits matmul.

Everything on-device works in a transposed [feature, token] layout so all
three matmuls consume weights in their natural [in, out] layout with tokens
as the moving operand; no on-chip transposes are needed.
"""

import numpy as np
import ml_dtypes

import concourse.bass as bass
import concourse.bacc as bacc
import concourse.mybir as mybir
from concourse.tile import TileContext
from concourse.bass_utils import run_bass_kernel_spmd

B, S, H = 4, 2048, 1024
F = 4 * H            # 4096
E = 8
TOP_K = 2
T = B * S            # 8192
P = 128
HK = H // P          # 8
FM = F // P          # 32
TSLICE = T // E      # 1024 router-logit tokens per core

# Per-expert token capacity. Seed-0 inputs route at most 2182 tokens to one
# expert; 2304 = 9*256 leaves margin and keeps 256-wide matmul tiles legal.
C = 2304
OUTER = [(0, 768), (768, 768), (1536, 768)]
CHUNK = 768

_NC = None
LAST_RESULTS = None


def _build_bass():
    nc = bacc.Bacc("TRN2", target_bir_lowering=False)
    f32 = mybir.dt.float32
    f32r = mybir.dt.float32r
    bf16 = mybir.dt.bfloat16
    Silu = mybir.ActivationFunctionType.Silu

    xT = nc.dram_tensor("xT", [H, C], f32r, kind="ExternalInput")
    wg = nc.dram_tensor("wg", [H, F], f32r, kind="ExternalInput")
    wu = nc.dram_tensor("wu", [H, F], f32r, kind="ExternalInput")
    wd = nc.dram_tensor("wd", [F, H], bf16, kind="ExternalInput")
    cw = nc.dram_tensor("cw", [P, C], f32, kind="ExternalInput")
    xTs = nc.dram_tensor("xTs", [H, TSLICE], f32, kind="ExternalInput")
    wgr = nc.dram_tensor("wgr", [H, E], f32, kind="ExternalInput")
    yT = nc.dram_tensor("yT", [H, C], f32, kind="ExternalOutput")
    lgT = nc.dram_tensor("lgT", [E, TSLICE], f32, kind="ExternalOutput")

    xT_r = xT.rearrange("(k p) t -> p k t", p=P)
    wg_r = wg.rearrange("(k p) f -> p k f", p=P)
    wu_r = wu.rearrange("(k p) f -> p k f", p=P)
    wd_r = wd.rearrange("(m p) h -> p m h", p=P)
    xTs_r = xTs.rearrange("(k p) t -> p k t", p=P)
    wgr_r = wgr.rearrange("(k p) e -> p k e", p=P)
    yT_r = yT.rearrange("(j p) t -> p j t", p=P)

    with TileContext(nc) as tc:
        # SwiGLU FFN over the dispatched token batch, in [feature, token]
        # layout: h = silu(wg.T @ xT) * (wu.T @ xT); yT = (wd.T @ h) * cw.
        with (
            tc.tile_pool(name="xp", bufs=1) as xp,
            tc.tile_pool(name="wp", bufs=3) as wp,
            tc.tile_pool(name="hp", bufs=1) as hp,
            tc.tile_pool(name="sp", bufs=3) as sp,
            tc.tile_pool(name="op", bufs=3) as op,
            tc.tile_pool(name="cwp", bufs=1) as cwp,
            tc.tile_pool(name="gups", bufs=2, space="PSUM") as gups,
            tc.tile_pool(name="ops", bufs=2, space="PSUM") as ops,
            tc.tile_pool(name="router", bufs=1) as rp,
            tc.tile_pool(name="router_ps", bufs=2, space="PSUM") as rps,
        ):
            cw_sb = cwp.tile([P, C], f32, tag="cw")
            nc.sync.dma_start(out=cw_sb, in_=cw.ap())

            # Router logits for this core's 1024-token slice: lgT = wgr.T @
            # xTs. Issued in the same scope as the FFN so Tile can overlap its
            # DMAs and (slow, fp32) matmuls with the FFN stream.
            wgr_sb = rp.tile([P, HK, E], f32, tag="wgr")
            nc.sync.dma_start(out=wgr_sb, in_=wgr_r)
            for n0 in range(0, TSLICE, 512):
                xTs_sb = rp.tile([P, HK, 512], f32, tag="xTs", name="xTs_sb")
                nc.sync.dma_start(out=xTs_sb, in_=xTs_r[:, :, n0 : n0 + 512])
                ps = rps.tile([E, 512], f32, tag="rps", name="ps")
                for k in range(HK):
                    nc.tensor.matmul(
                        ps,
                        lhsT=wgr_sb[:, k, :],
                        rhs=xTs_sb[:, k, :],
                        start=(k == 0),
                        stop=(k == HK - 1),
                    )
                lsb = rp.tile([E, 512], f32, tag="lsb", name="lsb")
                nc.vector.tensor_copy(out=lsb, in_=ps)
                nc.sync.dma_start(out=lgT[:, n0 : n0 + 512], in_=lsb)

            # wd is small enough (bf16, 16KB/partition) to keep resident for
            # the whole kernel: one well-formed 8MB DMA instead of 2.5x8
            # strided 256B-burst reloads, and phase 2 never waits on DMA.
            wd_sb = cwp.tile([P, FM, H], bf16, tag="wdres")
            nc.sync.dma_start(out=wd_sb, in_=wd_r)

            for c0, cn in OUTER:
                ntiles = [(o, min(512, cn - o)) for o in range(0, cn, 512)]
                xT_sb = xp.tile([P, HK, CHUNK], f32r, tag="xT")
                nc.sync.dma_start(out=xT_sb[:, :, :cn], in_=xT_r[:, :, c0 : c0 + cn])
                h_sb = hp.tile([P, FM, CHUNK], bf16, tag="h")

                for m in range(FM):
                    wg_sb = wp.tile([P, HK, P], f32r, tag="wg")
                    nc.sync.dma_start(out=wg_sb, in_=wg_r[:, :, m * P : (m + 1) * P])
                    wu_sb = wp.tile([P, HK, P], f32r, tag="wu")
                    nc.sync.dma_start(out=wu_sb, in_=wu_r[:, :, m * P : (m + 1) * P])
                    for no, nn in ntiles:
                        gps = gups.tile([P, 512], f32, tag="g", name="gps")[:, :nn]
                        ups = gups.tile([P, 512], f32, tag="u", name="ups")[:, :nn]
                        for k in range(HK):
                            nc.tensor.matmul(
                                gps,
                                lhsT=wg_sb[:, k, :],
                                rhs=xT_sb[:, k, no : no + nn],
                                start=(k == 0),
                                stop=(k == HK - 1),
                            )
                        for k in range(HK):
                            nc.tensor.matmul(
                                ups,
                                lhsT=wu_sb[:, k, :],
                                rhs=xT_sb[:, k, no : no + nn],
                                start=(k == 0),
                                stop=(k == HK - 1),
                            )
                        sg = sp.tile([P, 512], f32, tag="sg", name="sg")[:, :nn]
                        nc.scalar.activation(sg, gps, Silu)
                        nc.vector.tensor_mul(
                            out=h_sb[:, m, no : no + nn], in0=sg, in1=ups
                        )

                for j in range(HK):
                    for no, nn in ntiles:
                        ops_t = ops.tile([P, 512], f32, tag="o", name="opst")[:, :nn]
                        for m in range(FM):
                            nc.tensor.matmul(
                                ops_t,
                                lhsT=wd_sb[:, m, j * P : (j + 1) * P],
                                rhs=h_sb[:, m, no : no + nn],
                                start=(m == 0),
                                stop=(m == FM - 1),
                            )
                        ob = op.tile([P, 512], f32, tag="ob", name="ob")[:, :nn]
                        nc.vector.tensor_mul(
                            out=ob, in0=ops_t, in1=cw_sb[:, c0 + no : c0 + no + nn]
                        )
                        nc.sync.dma_start(
                            out=yT_r[:, j, c0 + no : c0 + no + nn], in_=ob
                        )
    nc.compile()
    return nc


def _get_nc():
    global _NC
    if _NC is None:
        _NC = _build_bass()
    return _NC


def kernel(hidden_states, w_gate, w_gate_proj, w_up_proj, w_down_proj):
    global LAST_RESULTS
    x = np.ascontiguousarray(np.asarray(hidden_states, dtype=np.float32).reshape(T, H))
    wgate = np.ascontiguousarray(np.asarray(w_gate, dtype=np.float32))

    # Routing decision (host — determines the expert-parallel dispatch).
    logits = x @ wgate
    mx = logits.max(-1, keepdims=True)
    ex = np.exp(logits - mx)
    probs = ex / ex.sum(-1, keepdims=True)
    top_i = np.argsort(-probs, axis=-1, kind="stable")[:, :TOP_K]
    top_w = np.take_along_axis(probs, top_i, axis=-1)
    top_w = top_w / top_w.sum(-1, keepdims=True)

    in_maps = []
    idx_list = []
    cnt_list = []
    for e in range(E):
        sel = top_i == e
        tok = np.nonzero(sel.any(-1))[0]
        cnt = len(tok)
        assert cnt <= C, f"expert {e} overflows capacity: {cnt} > {C}"
        w_tok = np.where(sel[tok, 0], top_w[tok, 0], top_w[tok, 1]).astype(np.float32)
        xTe = np.zeros((H, C), np.float32)
        xTe[:, :cnt] = x[tok].T
        cwv = np.zeros((C,), np.float32)
        cwv[:cnt] = w_tok
        in_maps.append(
            {
                "xT": xTe,
                "wg": np.ascontiguousarray(np.asarray(w_gate_proj[e], np.float32)),
                "wu": np.ascontiguousarray(np.asarray(w_up_proj[e], np.float32)),
                "wd": np.ascontiguousarray(np.asarray(w_down_proj[e], np.float32)).astype(
                    ml_dtypes.bfloat16
                ),
                "cw": np.ascontiguousarray(np.broadcast_to(cwv, (P, C))),
                "xTs": np.ascontiguousarray(x[e * TSLICE : (e + 1) * TSLICE].T),
                "wgr": wgate,
            }
        )
        idx_list.append(tok)
        cnt_list.append(cnt)

    nc = _get_nc()
    res = run_bass_kernel_spmd(nc, in_maps, core_ids=list(range(E)))
    LAST_RESULTS = res

    out = np.zeros((T, H), np.float32)
    lg = np.zeros((T, E), np.float32)
    for e in range(E):
        yTe = np.asarray(res.results[e]["yT"])  # [H, C]
        out[idx_list[e]] += yTe.T[: cnt_list[e]]
        lg[e * TSLICE : (e + 1) * TSLICE] = np.asarray(res.results[e]["lgT"]).T
    return out.reshape(B, S, H), lg


# revision 29
# speedup vs baseline: 1.1188x; 1.0118x over previous
"""Trainium2 Bass kernel for an 8-expert top-2 MoE SwiGLU FFN layer.

Sharding: expert-parallel over 8 NeuronCores (one expert per core).
Host side (the shard/unshard steps): token routing (softmax top-2) to build
the per-expert dispatch, gather/pad token batches to a fixed capacity,
scatter-add the weighted expert outputs back. Device side: per-expert dense
SwiGLU FFN over the dispatched tokens plus the router-logits matmul.

Everything on-device works in a transposed [feature, token] layout so all
three matmuls consume weights in their natural [in, out] layout with tokens
as the moving operand; no on-chip transposes are needed.
"""

import numpy as np
import ml_dtypes

import concourse.bass as bass
import concourse.bacc as bacc
import concourse.mybir as mybir
from concourse.tile import TileContext
from concourse.bass_utils import run_bass_kernel_spmd

B, S, H = 4, 2048, 1024
F = 4 * H            # 4096
E = 8
TOP_K = 2
T = B * S            # 8192
P = 128
HK = H // P          # 8
FM = F // P          # 32
TSLICE = T // E      # 1024 router-logit tokens per core

# Per-expert token capacity. Seed-0 inputs route at most 2182 tokens to one
# expert; 2304 = 9*256 leaves margin and keeps 256-wide matmul tiles legal.
# If the harness's routing ever exceeds this, kernel() rebuilds the module
# with a bumped capacity instead of failing.
C_DEFAULT = 2304
CHUNK = 768

_NC_CACHE = {}
LAST_RESULTS = None


def _outer_chunks(cap):
    """Split cap into CHUNK-sized outer chunks; every chunk is a multiple of
    256 (so each matmul N-tile is 512 or 256 — float32r needs >=256 for full
    rate) and at most CHUNK (so the h tile fits SBUF)."""
    assert cap % 256 == 0 and cap > 0
    chunks, off = [], 0
    while cap - off > CHUNK:
        chunks.append((off, CHUNK))
        off += CHUNK
    chunks.append((off, cap - off))
    return chunks


def _build_bass(cap):
    nc = bacc.Bacc("TRN2", target_bir_lowering=False)
    f32 = mybir.dt.float32
    f32r = mybir.dt.float32r
    bf16 = mybir.dt.bfloat16
    Silu = mybir.ActivationFunctionType.Silu

    outer = _outer_chunks(cap)
    xT = nc.dram_tensor("xT", [H, cap], f32r, kind="ExternalInput")
    wg = nc.dram_tensor("wg", [H, F], f32r, kind="ExternalInput")
    wu = nc.dram_tensor("wu", [H, F], f32r, kind="ExternalInput")
    wd = nc.dram_tensor("wd", [F, H], bf16, kind="ExternalInput")
    cw = nc.dram_tensor("cw", [P, cap], f32, kind="ExternalInput")
    xTs = nc.dram_tensor("xTs", [H, TSLICE], f32, kind="ExternalInput")
    wgr = nc.dram_tensor("wgr", [H, E], f32, kind="ExternalInput")
    yT = nc.dram_tensor("yT", [H, cap], f32, kind="ExternalOutput")
    lgT = nc.dram_tensor("lgT", [E, TSLICE], f32, kind="ExternalOutput")

    xT_r = xT.rearrange("(k p) t -> p k t", p=P)
    wg_r = wg.rearrange("(k p) f -> p k f", p=P)
    wu_r = wu.rearrange("(k p) f -> p k f", p=P)
    wd_r = wd.rearrange("(m p) h -> p m h", p=P)
    xTs_r = xTs.rearrange("(k p) t -> p k t", p=P)
    wgr_r = wgr.rearrange("(k p) e -> p k e", p=P)
    yT_r = yT.rearrange("(j p) t -> p j t", p=P)

    with TileContext(nc) as tc:
        # SwiGLU FFN over the dispatched token batch, in [feature, token]
        # layout: h = silu(wg.T @ xT) * (wu.T @ xT); yT = (wd.T @ h) * cw.
        with (
            tc.tile_pool(name="xp", bufs=1) as xp,
            tc.tile_pool(name="wp", bufs=3) as wp,
            tc.tile_pool(name="hp", bufs=1) as hp,
            tc.tile_pool(name="sp", bufs=3) as sp,
            tc.tile_pool(name="op", bufs=3) as op,
            tc.tile_pool(name="cwp", bufs=1) as cwp,
            tc.tile_pool(name="gups", bufs=3, space="PSUM") as gups,
            tc.tile_pool(name="ops", bufs=2, space="PSUM") as ops,
            tc.tile_pool(name="router", bufs=1) as rp,
        ):
            rps = ops  # router PSUM shares the down-matmul pool's banks
            cw_sb = cwp.tile([P, cap], f32, tag="cw")

            # Router logits for this core's 1024-token slice: lgT = wgr.T @
            # xTs. Issued in the same scope as the FFN so Tile can overlap its
            # DMAs and (slow, fp32) matmuls with the FFN stream.
            wgr_sb = cwp.tile([P, HK, E], f32, tag="wgr")
            nc.sync.dma_start(out=wgr_sb, in_=wgr_r)
            for n0 in range(0, TSLICE, 512):
                xTs_sb = rp.tile([P, HK, 512], f32, tag="xTs", name="xTs_sb")
                nc.sync.dma_start(out=xTs_sb, in_=xTs_r[:, :, n0 : n0 + 512])
                ps = rps.tile([E, 512], f32, tag="o", name="ps")
                for k in range(HK):
                    nc.tensor.matmul(
                        ps,
                        lhsT=wgr_sb[:, k, :],
                        rhs=xTs_sb[:, k, :],
                        start=(k == 0),
                        stop=(k == HK - 1),
                    )
                lsb = op.tile([E, 512], f32, tag="ob", name="lsb")
                nc.vector.tensor_copy(out=lsb, in_=ps)
                nc.sync.dma_start(out=lgT[:, n0 : n0 + 512], in_=lsb)

            # wd is small enough (bf16, 16KB/partition) to keep resident for
            # the whole kernel: one well-formed 8MB DMA instead of 2.5x8
            # strided 256B-burst reloads, and phase 2 never waits on DMA. The
            # wd/cw loads are issued after chunk-0 phase 1 (first use is
            # phase 2) so they don't hog the DMA queues at startup.
            wd_sb = cwp.tile([P, FM, H], bf16, tag="wdres")

            for ci, (c0, cn) in enumerate(outer):
                ntiles = [(o, min(512, cn - o)) for o in range(0, cn, 512)]
                xT_sb = xp.tile([P, HK, CHUNK], f32r, tag="xT")
                nc.sync.dma_start(out=xT_sb[:, :, :cn], in_=xT_r[:, :, c0 : c0 + cn])
                h_sb = hp.tile([P, FM, CHUNK], bf16, tag="h")

                for m in range(FM):
                    wg_sb = wp.tile([P, HK, P], f32r, tag="wg")
                    nc.sync.dma_start(out=wg_sb, in_=wg_r[:, :, m * P : (m + 1) * P])
                    wu_sb = wp.tile([P, HK, P], f32r, tag="wu")
                    nc.sync.dma_start(out=wu_sb, in_=wu_r[:, :, m * P : (m + 1) * P])
                    for no, nn in ntiles:
                        gps = gups.tile([P, 512], f32, tag="g", name="gps")[:, :nn]
                        ups = gups.tile([P, 512], f32, tag="u", name="ups")[:, :nn]
                        for k in range(HK):
                            nc.tensor.matmul(
                                gps,
                                lhsT=wg_sb[:, k, :],
                                rhs=xT_sb[:, k, no : no + nn],
                                start=(k == 0),
                                stop=(k == HK - 1),
                            )
                        for k in range(HK):
                            nc.tensor.matmul(
                                ups,
                                lhsT=wu_sb[:, k, :],
                                rhs=xT_sb[:, k, no : no + nn],
                                start=(k == 0),
                                stop=(k == HK - 1),
                            )
                        sg = sp.tile([P, 512], f32, tag="sg", name="sg")[:, :nn]
                        nc.scalar.activation(sg, gps, Silu)
                        nc.vector.tensor_mul(
                            out=h_sb[:, m, no : no + nn], in0=sg, in1=ups
                        )

                if ci == 0:
                    nc.sync.dma_start(out=wd_sb, in_=wd_r)
                    nc.sync.dma_start(out=cw_sb, in_=cw.ap())

                for j in range(HK):
                    for no, nn in ntiles:
                        ops_t = ops.tile([P, 512], f32, tag="o", name="opst")[:, :nn]
                        for m in range(FM):
                            nc.tensor.matmul(
                                ops_t,
                                lhsT=wd_sb[:, m, j * P : (j + 1) * P],
                                rhs=h_sb[:, m, no : no + nn],
                                start=(m == 0),
                                stop=(m == FM - 1),
                            )
                        ob = op.tile([P, 512], f32, tag="ob", name="ob")[:, :nn]
                        nc.vector.tensor_mul(
                            out=ob, in0=ops_t, in1=cw_sb[:, c0 + no : c0 + no + nn]
                        )
                        nc.sync.dma_start(
                            out=yT_r[:, j, c0 + no : c0 + no + nn], in_=ob
                        )
    nc.compile()
    return nc


def _get_nc(cap=C_DEFAULT):
    if cap not in _NC_CACHE:
        _NC_CACHE[cap] = _build_bass(cap)
    return _NC_CACHE[cap]


def kernel(hidden_states, w_gate, w_gate_proj, w_up_proj, w_down_proj):
    global LAST_RESULTS
    x = np.ascontiguousarray(np.asarray(hidden_states, dtype=np.float32).reshape(T, H))
    wgate = np.ascontiguousarray(np.asarray(w_gate, dtype=np.float32))

    # Routing decision (host — determines the expert-parallel dispatch).
    logits = x @ wgate
    mx = logits.max(-1, keepdims=True)
    ex = np.exp(logits - mx)
    probs = ex / ex.sum(-1, keepdims=True)
    top_i = np.argsort(-probs, axis=-1, kind="stable")[:, :TOP_K]
    top_w = np.take_along_axis(probs, top_i, axis=-1)
    top_w = top_w / top_w.sum(-1, keepdims=True)

    idx_list = [np.nonzero((top_i == e).any(-1))[0] for e in range(E)]
    cnt_list = [len(t) for t in idx_list]
    cap = C_DEFAULT
    if max(cnt_list) > cap:
        cap = -(-max(cnt_list) // 256) * 256  # round up to a 256 multiple

    in_maps = []
    for e in range(E):
        sel = top_i == e
        tok = idx_list[e]
        cnt = cnt_list[e]
        w_tok = np.where(sel[tok, 0], top_w[tok, 0], top_w[tok, 1]).astype(np.float32)
        xTe = np.zeros((H, cap), np.float32)
        xTe[:, :cnt] = x[tok].T
        cwv = np.zeros((cap,), np.float32)
        cwv[:cnt] = w_tok
        in_maps.append(
            {
                "xT": xTe,
                "wg": np.ascontiguousarray(np.asarray(w_gate_proj[e], np.float32)),
                "wu": np.ascontiguousarray(np.asarray(w_up_proj[e], np.float32)),
                "wd": np.ascontiguousarray(np.asarray(w_down_proj[e], np.float32)).astype(
                    ml_dtypes.bfloat16
                ),
                "cw": np.ascontiguousarray(np.broadcast_to(cwv, (P, cap))),
                "xTs": np.ascontiguousarray(x[e * TSLICE : (e + 1) * TSLICE].T),
                "wgr": wgate,
            }
        )

    nc = _get_nc(cap)
    res = run_bass_kernel_spmd(nc, in_maps, core_ids=list(range(E)))
    LAST_RESULTS = res

    out = np.zeros((T, H), np.float32)
    lg = np.zeros((T, E), np.float32)
    for e in range(E):
        yTe = np.asarray(res.results[e]["yT"])  # [H, C]
        out[idx_list[e]] += yTe.T[: cnt_list[e]]
        lg[e * TSLICE : (e + 1) * TSLICE] = np.asarray(res.results[e]["lgT"]).T
    return out.reshape(B, S, H), lg
